# revision 7
# baseline (speedup 1.0000x reference)
"""AdaArcFace loss on 8 TRN2 NeuronCores (Bass, class-sharded tensor parallel).

loss = mean_i( LSE_i - 32*cosm_i ),  LSE_i = 32 + ln(S_i + em_i - ey_i)
  S_i  = sum_c exp(32*(cos[i,c] - 1))   <- the only term needing the big matmul
  cos_y/quantile/margin path is tiny, exact fp32, replicated on every core.

Sharding: 100000 classes -> 8 cores x 12544 (44 pad rows = -features[0], whose
softmax contribution is ~1e-17 relative). kernel shard is transposed on host
(layout only) so DMA streams contiguous and the PE gets emb-on-partitions.

v3 (from v2's batch-on-partitions matmul layout):
 - cross-core S exchange via remote_dma_broadcast (XOR-relative peers, slot j
   holds the shard-sum of peer my^j; slots are summed so the per-receiver
   permutation is harmless). Replaces the ~28us collective + row-space finale.
 - finale entirely in column space [128,2]: no PE transposes, final batch mean
   via one ones-stationary matmul into bank-7 PSUM slack.
 - per-class 1/||w||: PE qmm -> ACT ln/exp -> SBUF->SBUF flatten DMA ->
   gpsimd partition_broadcast. No DRAM bounce, no 0-stride HBM broadcast reads.
 - consts (identity, ones, biases) come in via a host-packed input instead of
   gpsimd memset/affine_select.
 - margin path (quantile/mask/sin) is off every loop dependency; phase 0 only
   needs FT + WT tile 0.
"""

import math
import numpy as np

import concourse.bass as bass
import concourse.mybir as mybir
from concourse import library_config
from concourse.bass_utils import run_bass_kernel_spmd

F32 = mybir.dt.float32
F32R = mybir.dt.float32r
BF16 = mybir.dt.bfloat16

# problem constants (hardcoded per harness contract)
B = 256          # batch
E = 512          # embedding
C = 100000       # classes
NCORES = 8
CPC = 12544      # classes per core (padded): 98 groups of 128
TILE_C = 1792    # classes per stream tile: 14 groups of 128
NTILES = CPC // TILE_C             # 7
GPT = TILE_C // 128                # 14 groups per tile
NCH = 4                            # 448-wide class chunks per tile
CHW = TILE_C // NCH                # 448
NPH = 2 * NTILES                   # 14 phases (bc-halves)
SCALE = 32.0
M_BASE = 0.5
ALPHA = 0.1
BETA = 0.15
SIN_M = math.sin(M_BASE)
LN32 = math.log(32.0)
ECH = E // 128   # 4 emb chunks

_CACHE = {}


def build_nc():
    nc = bass.Bass(target_bir_lowering=False, num_devices=NCORES, num_swdge_queues=2)

    wt_ext = nc.declare_dram_parameter(
        "wt", [NTILES, ECH, 128, TILE_C], F32R, isOutput=False)
    feat_ext = nc.declare_dram_parameter("feat", [B, E], F32, isOutput=False)
    wlab_ext = nc.declare_dram_parameter("wlab", [B, E], F32, isOutput=False)
    cpack_ext = nc.declare_dram_parameter("cpack", [128, 132], F32, isOutput=False)
    cbf_ext = nc.declare_dram_parameter("cbf", [128, 1], BF16, isOutput=False)
    out_ext = nc.declare_dram_parameter("out", [1, 1], F32, isOutput=True)

    WT_TILE_ELEMS = ECH * 128 * TILE_C

    from contextlib import ExitStack
    ctx = ExitStack()
    sb = lambda name, shape, dt=F32: ctx.enter_context(nc.sbuf_tensor(name, shape, dt))
    ps = lambda name, shape, dt=F32: ctx.enter_context(nc.psum_tensor(name, shape, dt))
    sem = lambda name: ctx.enter_context(nc.semaphore(name))

    with ctx:
        # --- SBUF ---
        WT = [sb(f"WT{i}", [128, ECH, TILE_C], F32R) for i in range(3)]
        W2 = sb("W2", [128, ECH, TILE_C], BF16)
        W2P = sb("W2P", [128, 2, TILE_C], BF16)
        W2S = [sb(f"W2S{i}", [128, TILE_C], BF16) for i in range(2)]
        LNQ = sb("LNQ", [128, GPT])
        INVC = [sb(f"INVC{i}", [128, GPT]) for i in range(2)]   # 32/||w|| columns
        INVR = [sb(f"INVR{i}", [1, TILE_C]) for i in range(2)]  # flattened rows
        INVB = [sb(f"INVB{i}", [128, TILE_C]) for i in range(2)]  # broadcast
        CN = [sb(f"CN{i}", [128, TILE_C]) for i in range(2)]    # 32*cos, per phase
        EJ = [sb(f"EJ{i}", [128, TILE_C], BF16) for i in range(2)]  # exp out (junk)
        SACC = sb("SACC", [128, 2, NTILES])                     # accum_out slots
        FT = sb("FT", [128, ECH, 2, 128], F32R)                 # fhatT: [e_p, ec, bc, b]
        F_ = sb("F", [128, 2, E])
        FN = sb("FN", [128, 2, E])
        WL = sb("WL", [128, 2, E])
        WLN = sb("WLN", [128, 2, E])
        CMP2 = sb("CMP2", [128, B])
        DB = sb("DB", [128, B])        # difficulty broadcast (rank compares)
        DDR = sb("DDR", [1, B])        # difficulty flattened row
        AG = sb("AG", [128, 2 * NCORES])  # remote slot j at cols [2j, 2j+2)
        qfw = sb("qfw", [128, 4])
        lnq4 = sb("lnq4", [128, 4])
        invfw = sb("invfw", [128, 4])
        cosy = sb("cosy", [128, 2]); dd = sb("dd", [128, 2])
        cnt = sb("cnt", [128, 2]); mask = sb("mask", [128, 2])
        t1 = sb("t1", [128, 2]); m015 = sb("m015", [128, 2]); mm_ = sb("mm", [128, 2])
        cmpv = sb("cmpv", [128, 2]); t2 = sb("t2", [128, 2]); t3 = sb("t3", [128, 2])
        cosm = sb("cosm", [128, 2]); ey = sb("ey", [128, 2]); em = sb("em", [128, 2])
        adj = sb("adj", [128, 2]); li0c = sb("li0c", [128, 2])
        Sb = sb("Sb", [128, 2]); Sf1 = sb("Sf1", [128, 2])
        Sfull = sb("Sfull", [128, 2]); TT = sb("TT", [128, 2])
        lS = sb("lS", [128, 2]); li = sb("li", [128, 2])
        lsum = sb("lsum", [1, 1]); loss = sb("loss", [1, 1])
        cpack = sb("cpk", [128, 132])
        onesbf = sb("obf", [128, 1], BF16)

        # --- PSUM: one 16KB tensor, manually laid out ---
        # D ping: chunks at 512*k (448 wide), k=0..3   [banks 0-3]
        # D pong: 2048 + 512*k                          [banks 4-7]
        # qcol ping: [1984:1998) (bank-3 slack); pong: [4032:4046) (bank-7 slack)
        # FT staging (before phase 0): [0:256)
        # finale loss row: [4064:4066) (bank-7 slack)
        PS = ps("PS", [128, 4096])
        Doff = lambda ph, k: (ph % 2) * 2048 + 512 * k
        QOFF = [1984, 4032]
        FOFF = 4064

        # --- semaphores ---
        s_inF = sem("s_inF"); s_inW = sem("s_inW"); s_cst = sem("s_cst")
        s_wtb = [sem(f"s_wtb{i}") for i in range(3)]
        s_qfw = sem("s_qfw"); s_fn = sem("s_fn"); s_wln = sem("s_wln")
        s_sqA = sem("s_sqA"); s_sq = sem("s_sq")
        s_gpA = sem("s_gpA"); s_gp = sem("s_gp"); s_w2s = sem("s_w2s")
        s_qmm = sem("s_qmm"); s_invc = sem("s_invc"); s_fl = sem("s_fl")
        s_invb = sem("s_invb")
        s_D = sem("s_D"); s_mult = sem("s_mult"); s_exp = sem("s_exp")
        s_ftp = sem("s_ftp"); s_ftc = sem("s_ftc")
        s_cy = sem("s_cy"); s_ddf = sem("s_ddf"); s_db = sem("s_db")
        s_mask = sem("s_mask"); s_sin = sem("s_sin"); s_cosm = sem("s_cosm")
        s_eyem = sem("s_eyem")
        s_sb = sem("s_sb"); s_tt = sem("s_tt"); s_lns = sem("s_lns")
        s_li = sem("s_li"); s_fin = sem("s_fin"); s_loss = sem("s_loss")
        s_prep = sem("s_prep"); s_rls = sem("s_rls"); s_rs = sem("s_rs")
        s_gd = sem("s_gd")
        s_vh = sem("s_vh"); s_ah = sem("s_ah")

        _hs = {"v": 0, "a": 0}

        def vbar(eng, ins):
            key = "v" if eng.engine == mybir.EngineType.DVE else "a"
            s = s_vh if key == "v" else s_ah
            _hs[key] += 1
            ins.then_inc(s, 1)
            eng.wait_ge(s, _hs[key])

        with nc.Block() as block:

            # ---------------- SYNC: input DMAs ----------------
            @block.sync
            def _(sync):
                sync.dma_start(
                    F_[:, :, :],
                    bass.AP(feat_ext, 0, [[E, 128], [128 * E, 2], [1, E]]),
                ).then_inc(s_inF, 16)
                sync.dma_start(
                    WL[:, :, :],
                    bass.AP(wlab_ext, 0, [[E, 128], [128 * E, 2], [1, E]]),
                ).then_inc(s_inW, 16)
                sync.dma_start(cpack[:, :], cpack_ext.ap()).then_inc(s_cst, 16)
                sync.dma_start(onesbf[:, :], cbf_ext.ap()).then_inc(s_cst, 16)
                for t in range(NTILES):
                    if t >= 3:
                        sync.wait_ge(s_sq, t - 2)        # ACT squares of t-3 done
                        sync.wait_ge(s_D, 2 * (t - 2))   # PE phases of t-3 done
                    sync.dma_start(
                        WT[t % 3][:, :, :],
                        bass.AP(wt_ext, t * WT_TILE_ELEMS,
                                [[TILE_C, 128], [128 * TILE_C, ECH], [1, TILE_C]]),
                    ).then_inc(s_wtb[t % 3], 16)

            # ---------------- GPSIMD: bcasts, remote exchange, out ----------
            @block.gpsimd
            def _(g):
                g.load_library(library_config.proxy)
                # difficulty row broadcast for the rank/quantile compares
                g.wait_ge(s_cy, 1)
                g.dma_start(
                    bass.AP(DDR, 0, [[B, 1], [2, 128], [1, 2]]),
                    bass.AP(dd, 0, [[2, 128], [1, 2]]),
                ).then_inc(s_ddf, 16)
                g.wait_ge(s_ddf, 16)
                g.partition_broadcast(DB[:, :], DDR[0:1, :]).then_inc(s_db, 1)
                # remote S-exchange descriptors (addresses only; fired at end).
                # Round j sends Sb to peer my_tpb^j into AG slot j; receiver r's
                # slot j ends up with peer r^j's shard sum. Slots are summed, so
                # the sender permutation is irrelevant.
                for j in range(1, NCORES):
                    g.remote_dma_broadcast(
                        AG[:, 2 * j:2 * j + 2], Sb[:, :],
                        remote_sem=s_rs, local_sem=s_rls,
                        rdests=[(0, j) if k == j else None for k in range(NCORES)],
                        queue_num=1,
                    ).then_inc(s_prep, 1)

                # per-tile inv-norm flatten + broadcast
                for t in range(NTILES):
                    g.wait_ge(s_invc, t + 1)
                    g.dma_start(
                        bass.AP(INVR[t % 2], 0, [[TILE_C, 1], [GPT, 128], [1, GPT]]),
                        bass.AP(INVC[t % 2], 0, [[GPT, 128], [GPT, 1], [1, GPT]]),
                    ).then_inc(s_fl, 16)
                    g.wait_ge(s_fl, 16 * (t + 1))
                    if t >= 2:
                        g.wait_ge(s_mult, 2 * (t - 1))  # INVB buffer free
                    g.partition_broadcast(
                        INVB[t % 2][:, :], INVR[t % 2][0:1, :]).then_inc(s_invb, 1)

                # fire the S exchange once local shard sums are ready
                g.wait_ge(s_prep, NCORES - 1)
                g.wait_ge(s_sb, 1)
                g.trigger_dma(count=NCORES - 1, queue_num=1)
                g.wait_ge(s_loss, 1)
                g.dma_start(out_ext[:, :], loss[:, :]).then_inc(s_gd, 16)
                g.wait_ge(s_gd, 16)

            # ---------------- ACT (scalar) ----------------
            @block.scalar
            def _(a):
                Act = mybir.ActivationFunctionType
                halfpi = cpack[:, 129:130]
                neg32 = cpack[:, 130:131]
                ln32c = cpack[:, 131:132]
                # norms of f and wlab (squares -> ln/exp rsqrt, one act table)
                a.wait_ge(s_inF, 16)
                a.activation(CN[0][:, 0:E], F_[:, 0, :], Act.Square,
                             accum_out=qfw[:, 0:1])
                a.activation(CN[0][:, E:2 * E], F_[:, 1, :], Act.Square,
                             accum_out=qfw[:, 1:2])
                a.wait_ge(s_inW, 16)
                a.activation(CN[1][:, 0:E], WL[:, 0, :], Act.Square,
                             accum_out=qfw[:, 2:3])
                ins = a.activation(CN[1][:, E:2 * E], WL[:, 1, :], Act.Square,
                                   accum_out=qfw[:, 3:4])
                vbar(a, ins)
                ins = a.activation(lnq4[:, :], qfw[:, :], Act.Ln)
                vbar(a, ins)
                ins = a.activation(invfw[:, :], lnq4[:, :], Act.Exp, scale=-0.5)
                vbar(a, ins)
                a.activation(FN[:, 0, :], F_[:, 0, :], Act.Copy,
                             scale=invfw[:, 0:1])
                a.activation(FN[:, 1, :], F_[:, 1, :], Act.Copy,
                             scale=invfw[:, 1:2]).then_inc(s_fn, 1)
                a.activation(WLN[:, 0, :], WL[:, 0, :], Act.Copy,
                             scale=invfw[:, 2:3])
                a.activation(WLN[:, 1, :], WL[:, 1, :], Act.Copy,
                             scale=invfw[:, 3:4]).then_inc(s_wln, 1)
                # margin path: cos(m*pi) = -sin(m*pi - pi/2), arg in [0, 1.1]
                a.wait_ge(s_cst, 32)
                a.wait_ge(s_mask, 1)
                a.activation(cmpv[:, :], mm_[:, :], Act.Sin,
                             bias=halfpi, scale=math.pi).then_inc(s_sin, 1)
                a.wait_ge(s_cosm, 1)
                a.activation(ey[:, :], cosy[:, :], Act.Exp,
                             bias=neg32, scale=SCALE)
                a.activation(em[:, :], cosm[:, :], Act.Exp,
                             bias=neg32, scale=SCALE).then_inc(s_eyem, 1)

                def a_square(t):
                    a.wait_ge(s_wtb[t % 3], 16 * (t // 3 + 1))
                    if t >= 1:
                        a.wait_ge(s_gpA, t)       # presum1(t-1) done with W2[0:2]
                    a.activation(W2[:, 0:2, :],
                                 WT[t % 3][:, 0:2, :].bitcast(F32),
                                 Act.Square).then_inc(s_sqA, 1)
                    if t >= 1:
                        a.wait_ge(s_gp, t)        # presum2(t-1) done with W2[2:4]
                    a.activation(W2[:, 2:4, :],
                                 WT[t % 3][:, 2:4, :].bitcast(F32),
                                 Act.Square).then_inc(s_sq, 1)

                def a_inv(t):
                    a.wait_ge(s_qmm, t + 1)
                    if t >= 2:
                        a.wait_ge(s_fl, 16 * (t - 1))  # INVC buffer free
                    ins = a.activation(LNQ[:, :],
                                       PS[:, QOFF[t % 2]:QOFF[t % 2] + GPT],
                                       Act.Ln)
                    vbar(a, ins)
                    a.activation(INVC[t % 2][:, :], LNQ[:, :], Act.Exp,
                                 bias=ln32c, scale=-0.5).then_inc(s_invc, 1)

                def a_exp(ph):
                    t, half = ph // 2, ph % 2
                    a.wait_ge(s_mult, ph + 1)
                    if ph >= 2:
                        a.wait_ge(s_exp, ph - 1)  # EJ buffer visible-order
                    a.activation(
                        EJ[ph % 2][:, :], CN[ph % 2][:, :], Act.Exp,
                        bias=neg32,
                        accum_out=bass.AP(
                            SACC, half * NTILES + t,
                            [[2 * NTILES, 128], [1, 1]])).then_inc(s_exp, 1)

                for t in range(NTILES):
                    a_square(t)
                    if t >= 1:
                        a_exp(2 * (t - 1))
                        a_exp(2 * (t - 1) + 1)
                    a_inv(t)
                a_exp(2 * (NTILES - 1))
                a_exp(2 * (NTILES - 1) + 1)

                # finale
                a.wait_ge(s_tt, 1)
                a.activation(lS[:, :], TT[:, :], Act.Ln).then_inc(s_lns, 1)

            # ---------------- DVE (vector) ----------------
            @block.vector
            def _(v):
                Alu = mybir.AluOpType
                # fT chunk copies (ping-pong with PE transposes through PS[0:256))
                for ec in range(ECH):
                    v.wait_ge(s_ftp, ec + 1)
                    v.tensor_copy(
                        bass.AP(FT, ec * 256, [[ECH * 256, 128], [1, 256]]),
                        PS[:, 0:256]).then_inc(s_ftc, 1)
                # cos_y (exact fp32) and difficulty
                v.wait_ge(s_wln, 1)
                for b in range(2):
                    scrd = CN[b][:, 2 * E:3 * E]  # (128, 512) scratch
                    ins = v.tensor_mul(scrd, FN[:, b, :], WLN[:, b, :])
                    vbar(v, ins)
                    ins = v.tensor_reduce(cosy[:, b:b + 1], scrd,
                                          axis=mybir.AxisListType.X, op=Alu.add)
                    vbar(v, ins)
                v.tensor_scalar(dd[:, :], cosy[:, :], -1.0, 1.0,
                                Alu.mult, Alu.add).then_inc(s_cy, 1)
                # rank/quantile: cnt_i = #{j: d_j <= d_i}; mask = cnt >= 52
                v.wait_ge(s_db, 1)
                for b in range(2):
                    ins = v.tensor_scalar(
                        CMP2[:, :], DB[:, :], dd[:, b:b + 1], 0.0,
                        Alu.is_le, Alu.add, accum_out=cnt[:, b:b + 1])
                    vbar(v, ins)
                v.tensor_scalar(mask[:, :], cnt[:, :], 51.5, None, Alu.is_ge)
                ins = v.tensor_scalar(t1[:, :], dd[:, :], ALPHA, M_BASE,
                                      Alu.mult, Alu.add)
                vbar(v, ins)
                ins = v.tensor_scalar(m015[:, :], mask[:, :], BETA, None, Alu.mult)
                vbar(v, ins)
                v.tensor_add(mm_[:, :], t1[:, :], m015[:, :]).then_inc(s_mask, 1)
                v.wait_ge(s_sin, 1)
                v.tensor_mul(t2[:, :], cosy[:, :], cmpv[:, :])
                ins = v.tensor_scalar(t3[:, :], mm_[:, :], -SIN_M, None, Alu.mult)
                vbar(v, ins)
                v.tensor_sub(cosm[:, :], t3[:, :], t2[:, :]).then_inc(s_cosm, 1)
                v.wait_ge(s_cosm, 1)  # self-wait doubles as visibility barrier
                v.tensor_scalar(li0c[:, :], cosm[:, :], -SCALE, SCALE,
                                Alu.mult, Alu.add)
                v.wait_ge(s_eyem, 1)
                v.tensor_sub(adj[:, :], em[:, :], ey[:, :])

                # big loop: presums interleaved with phase multiplies
                def v_mult(ph):
                    t = ph // 2
                    v.wait_ge(s_D, ph + 1)
                    v.wait_ge(s_invb, t + 1)
                    if ph >= 2:
                        v.wait_ge(s_exp, ph - 1)   # CN buffer free
                    ins = None
                    for k in range(NCH):
                        ins = v.tensor_mul(
                            CN[ph % 2][:, k * CHW:(k + 1) * CHW],
                            PS[:, Doff(ph, k):Doff(ph, k) + CHW],
                            INVB[t % 2][:, k * CHW:(k + 1) * CHW])
                    ins.then_inc(s_mult, 1)

                for t in range(NTILES):
                    v.wait_ge(s_sqA, t + 1)
                    if t >= 1:
                        v.wait_ge(s_w2s, t)       # presum3(t-1) done with W2P
                    v.tensor_add(W2P[:, 0, :], W2[:, 0, :],
                                 W2[:, 1, :]).then_inc(s_gpA, 1)
                    v.wait_ge(s_sq, t + 1)
                    v.tensor_add(W2P[:, 1, :], W2[:, 2, :],
                                 W2[:, 3, :]).then_inc(s_gp, 1)
                    v.wait_ge(s_gp, t + 1)        # barrier: W2P halves visible
                    if t >= 2:
                        v.wait_ge(s_qmm, t - 1)   # W2S buffer free
                    v.tensor_add(W2S[t % 2][:, :], W2P[:, 0, :],
                                 W2P[:, 1, :]).then_inc(s_w2s, 1)
                    v_mult(2 * t)
                    v_mult(2 * t + 1)

                # finale: local shard sum, remote exchange, loss
                v.wait_ge(s_exp, NPH)
                ins = v.tensor_reduce(
                    Sb[:, :],
                    bass.AP(SACC, 0, [[2 * NTILES, 128], [NTILES, 2], [1, NTILES]]),
                    axis=mybir.AxisListType.X, op=Alu.add)
                ins.then_inc(s_sb, 1)
                v.wait_ge(s_rs, 2 * (NCORES - 1))   # all 7 peers' slots landed
                ins = v.tensor_reduce(
                    Sf1[:, :],
                    bass.AP(AG, 2, [[2 * NCORES, 128], [1, 2], [2, NCORES - 1]]),
                    axis=mybir.AxisListType.X, op=Alu.add)
                vbar(v, ins)
                ins = v.tensor_add(Sfull[:, :], Sf1[:, :], Sb[:, :])
                vbar(v, ins)
                v.tensor_add(TT[:, :], Sfull[:, :], adj[:, :]).then_inc(s_tt, 1)
                v.wait_ge(s_lns, 1)
                v.tensor_add(li[:, :], lS[:, :], li0c[:, :]).then_inc(s_li, 1)
                v.wait_ge(s_fin, 1)
                ins = v.tensor_reduce(lsum[:, :], PS[0:1, FOFF:FOFF + 2],
                                      axis=mybir.AxisListType.X, op=Alu.add)
                vbar(v, ins)
                v.tensor_scalar(loss[:, :], lsum[:, :], 1.0 / B, None,
                                Alu.mult).then_inc(s_loss, 1)

            # ---------------- PE (tensor) ----------------
            @block.tensor
            def _(te):
                te.wait_ge(s_cst, 32)
                te.wait_ge(s_fn, 1)
                ident = cpack[:, 0:128]
                onesf = cpack[:, 128:129]
                # fT = transpose(f_norm): [e_p, ec, bc, b] via PS[0:256)
                for ec in range(ECH):
                    if ec >= 1:
                        te.wait_ge(s_ftc, ec)
                    te.transpose(PS[:, 0:128],
                                 FN[:, 0, ec * 128:(ec + 1) * 128], ident)
                    te.transpose(PS[:, 128:256],
                                 FN[:, 1, ec * 128:(ec + 1) * 128],
                                 ident).then_inc(s_ftp, 1)

                def t_qmm(t):
                    # norm matmuls: q_g = ones^T @ W2S strided slice (bf16 FWL)
                    te.wait_ge(s_w2s, t + 1)
                    if t >= 2:
                        te.wait_ge(s_invc, t - 1)   # qcol slack reuse
                    ins = None
                    for gi in range(GPT):
                        # strided class slice {GPT*p + gi}: column p of the
                        # norm output is class GPT*p+gi, so the (p,g) flatten
                        # lands in natural class order
                        ins = te.matmul(
                            PS[:, QOFF[t % 2] + gi:QOFF[t % 2] + gi + 1],
                            bass.AP(W2S[t % 2], gi, [[TILE_C, 128], [GPT, 128]]),
                            onesbf[:, :])
                    ins.then_inc(s_qmm, 1)

                def t_phase(ph):
                    t, half = ph // 2, ph % 2
                    if half == 0:
                        te.wait_ge(s_wtb[t % 3], 16 * (t // 3 + 1))
                    if ph >= 2:
                        te.wait_ge(s_mult, ph - 1)  # D bank-set free
                    ins = None
                    for ec in range(ECH):
                        for k in range(NCH):
                            ins = te.matmul(
                                PS[:, Doff(ph, k):Doff(ph, k) + CHW],
                                FT[:, ec, half, :],
                                WT[t % 3][:, ec, k * CHW:(k + 1) * CHW],
                                start=(ec == 0), stop=(ec == ECH - 1),
                                skip_group_check=True)
                    ins.then_inc(s_D, 1)

                for t in range(NTILES):
                    if t == 0:
                        te.wait_ge(s_ftc, ECH)
                        t_phase(0)
                        t_phase(1)
                        t_qmm(0)
                    else:
                        t_phase(2 * t)
                        t_qmm(t)
                        t_phase(2 * t + 1)

                # finale: batch mean via ones-stationary matmul into bank-7 slack
                te.wait_ge(s_li, 1)
                te.matmul(PS[0:1, FOFF:FOFF + 2], onesf, li[:, :]).then_inc(s_fin, 1)

        return nc


def _shard_host(features, labels, kernel_w):
    """Host-side shard + pack (layout only, no arithmetic)."""
    features = np.ascontiguousarray(features, dtype=np.float32)
    kernel_w = np.ascontiguousarray(kernel_w, dtype=np.float32)
    labels = np.asarray(labels).astype(np.int64)
    wlab = np.ascontiguousarray(kernel_w[labels])        # (B, E) gather
    pad_row = -features[0]                               # direction only matters
    cpack = np.zeros((128, 132), dtype=np.float32)
    cpack[:, 0:128] = np.eye(128, dtype=np.float32)
    cpack[:, 128] = 1.0
    cpack[:, 129] = -math.pi / 2.0
    cpack[:, 130] = -SCALE
    cpack[:, 131] = LN32
    cbf = np.ones((128, 1), dtype=np.float32)  # cast to bf16 below
    try:
        import ml_dtypes
        cbf = cbf.astype(ml_dtypes.bfloat16)
    except ImportError:
        cbf = cbf.astype(np.uint16)  # fallback: bf16(1.0) = 0x3F80
        cbf[:] = 0x3F80
    in_maps = []
    cpc_raw = C // NCORES                                # 12500
    for c in range(NCORES):
        shard = kernel_w[c * cpc_raw:(c + 1) * cpc_raw]  # (12500, E)
        pad = np.broadcast_to(pad_row, (CPC - cpc_raw, E))
        shard = np.concatenate([shard, pad], axis=0)     # (12544, E)
        # (CPC, E) -> transpose -> (E, CPC) -> (ECH,128, NTILES,TILE_C)
        wt = shard.T.reshape(ECH, 128, NTILES, TILE_C)
        wt = np.ascontiguousarray(wt.transpose(2, 0, 1, 3))  # (NTILES,ECH,128,TILE_C)
        in_maps.append({"wt": wt, "feat": features, "wlab": wlab,
                        "cpack": cpack, "cbf": cbf})
    return in_maps


def _get_nc():
    if "nc" not in _CACHE:
        nc = build_nc()
        # populate .instr bytes for extended-inst InstISA ops
        # (partition_broadcast / remote_dma_broadcast / trigger_dma);
        # raw Bass doesn't run this pass and walrus errors on empty .instr.
        from concourse.library_overlay import lower_extended_insts
        lower_extended_insts(nc)
        _CACHE["nc"] = nc
    return _CACHE["nc"]


def kernel(features, labels, kernel):
    in_maps = _shard_host(features, labels, kernel)
    nc = _get_nc()
    res = run_bass_kernel_spmd(nc, in_maps, core_ids=list(range(NCORES)))
    out = res.results[0]["out"]
    return np.float32(out.reshape(())[()])


# revision 8
# speedup vs baseline: 62.2182x; 62.2182x over previous
"""AdaArcFace loss on 8 TRN2 NeuronCores (Bass, class-sharded tensor parallel).

loss = mean_i( LSE_i - 32*cosm_i ),  LSE_i = 32 + ln(S_i + em_i - ey_i)
  S_i  = sum_c exp(32*(cos[i,c] - 1))   <- the only term needing the big matmul
  cos_y/quantile/margin path is tiny, exact fp32, replicated on every core.

Sharding: 100000 classes -> 8 cores x 12544 (44 pad rows = -features[0], whose
softmax contribution is ~1e-17 relative). kernel shard is transposed on host
(layout only) so DMA streams contiguous and the PE gets emb-on-partitions.

v3 (from v2's batch-on-partitions matmul layout):
 - cross-core S exchange via remote_dma_broadcast (XOR-relative peers, slot j
   holds the shard-sum of peer my^j; slots are summed so the per-receiver
   permutation is harmless). Replaces the ~28us collective + row-space finale.
 - finale entirely in column space [128,2]: no PE transposes, final batch mean
   via one ones-stationary matmul into bank-7 PSUM slack.
 - per-class 1/||w||: PE qmm -> ACT ln/exp -> SBUF->SBUF flatten DMA ->
   gpsimd partition_broadcast. No DRAM bounce, no 0-stride HBM broadcast reads.
 - consts (identity, ones, biases) come in via a host-packed input instead of
   gpsimd memset/affine_select.
 - margin path (quantile/mask/sin) is off every loop dependency; phase 0 only
   needs FT + WT tile 0.
"""

import math
import numpy as np

import concourse.bass as bass
import concourse.mybir as mybir
from concourse import library_config
from concourse.bass_utils import run_bass_kernel_spmd

F32 = mybir.dt.float32
F32R = mybir.dt.float32r
BF16 = mybir.dt.bfloat16

# problem constants (hardcoded per harness contract)
B = 256          # batch
E = 512          # embedding
C = 100000       # classes
NCORES = 8
CPC = 12544      # classes per core (padded): 98 groups of 128
TILE_C = 1792    # classes per stream tile: 14 groups of 128
NTILES = CPC // TILE_C             # 7
GPT = TILE_C // 128                # 14 groups per tile
NCH = 4                            # 448-wide class chunks per tile
CHW = TILE_C // NCH                # 448
NPH = 2 * NTILES                   # 14 phases (bc-halves)
SCALE = 32.0
M_BASE = 0.5
ALPHA = 0.1
BETA = 0.15
SIN_M = math.sin(M_BASE)
LN32 = math.log(32.0)
ECH = E // 128   # 4 emb chunks

_CACHE = {}


def build_nc():
    nc = bass.Bass(target_bir_lowering=False, num_devices=NCORES, num_swdge_queues=2)

    wt_ext = nc.declare_dram_parameter(
        "wt", [NTILES, ECH, 128, TILE_C], F32R, isOutput=False)
    feat_ext = nc.declare_dram_parameter("feat", [B, E], F32, isOutput=False)
    wlab_ext = nc.declare_dram_parameter("wlab", [B, E], F32, isOutput=False)
    cpack_ext = nc.declare_dram_parameter("cpack", [128, 132], F32, isOutput=False)
    cbf_ext = nc.declare_dram_parameter("cbf", [128, 1], BF16, isOutput=False)
    out_ext = nc.declare_dram_parameter("out", [1, 1], F32, isOutput=True)

    # dummy collective: forces the runtime's synchronized multi-core launch
    # (without any collective in the NEFF the 8 cores start ms apart and the
    # remote S-exchange stalls on the slowest core)
    cc_in = nc.dram_tensor("cc_in", [1, 1], F32)
    cc_out = nc.dram_tensor("cc_out", [1, NCORES], F32, addr_space="Shared")

    WT_TILE_ELEMS = ECH * 128 * TILE_C

    from contextlib import ExitStack
    ctx = ExitStack()
    sb = lambda name, shape, dt=F32: ctx.enter_context(nc.sbuf_tensor(name, shape, dt))
    ps = lambda name, shape, dt=F32: ctx.enter_context(nc.psum_tensor(name, shape, dt))
    sem = lambda name: ctx.enter_context(nc.semaphore(name))

    with ctx:
        # --- SBUF ---
        WT = [sb(f"WT{i}", [128, ECH, TILE_C], F32R) for i in range(3)]
        W2 = sb("W2", [128, ECH, TILE_C], BF16)
        W2P = sb("W2P", [128, 2, TILE_C], BF16)
        W2S = [sb(f"W2S{i}", [128, TILE_C], BF16) for i in range(2)]
        LNQ = sb("LNQ", [128, GPT])
        INVC = [sb(f"INVC{i}", [128, GPT]) for i in range(2)]   # 32/||w|| columns
        INVR = [sb(f"INVR{i}", [1, TILE_C]) for i in range(2)]  # flattened rows
        INVB = [sb(f"INVB{i}", [128, TILE_C]) for i in range(2)]  # broadcast
        CN = [sb(f"CN{i}", [128, TILE_C]) for i in range(2)]    # 32*cos, per phase
        EJ = [sb(f"EJ{i}", [128, TILE_C], BF16) for i in range(2)]  # exp out (junk)
        SACC = sb("SACC", [128, 2, NTILES])                     # accum_out slots
        FT = sb("FT", [128, ECH, 2, 128], F32R)                 # fhatT: [e_p, ec, bc, b]
        F_ = sb("F", [128, 2, E])
        FN = sb("FN", [128, 2, E])
        WL = sb("WL", [128, 2, E])
        WLN = sb("WLN", [128, 2, E])
        CMP2 = sb("CMP2", [128, B])
        DB = sb("DB", [128, B])        # difficulty broadcast (rank compares)
        DDR = sb("DDR", [1, B])        # difficulty flattened row
        AG = sb("AG", [128, 2 * NCORES])  # remote slot j at cols [2j, 2j+2)
        qfw = sb("qfw", [128, 4])
        lnq4 = sb("lnq4", [128, 4])
        invfw = sb("invfw", [128, 4])
        cosy = sb("cosy", [128, 2]); dd = sb("dd", [128, 2])
        cnt = sb("cnt", [128, 2]); mask = sb("mask", [128, 2])
        t1 = sb("t1", [128, 2]); m015 = sb("m015", [128, 2]); mm_ = sb("mm", [128, 2])
        cmpv = sb("cmpv", [128, 2]); t2 = sb("t2", [128, 2]); t3 = sb("t3", [128, 2])
        cosm = sb("cosm", [128, 2]); ey = sb("ey", [128, 2]); em = sb("em", [128, 2])
        adj = sb("adj", [128, 2]); li0c = sb("li0c", [128, 2])
        Sb = sb("Sb", [128, 2]); Sf1 = sb("Sf1", [128, 2])
        Sfull = sb("Sfull", [128, 2]); TT = sb("TT", [128, 2])
        lS = sb("lS", [128, 2]); li = sb("li", [128, 2])
        lsum = sb("lsum", [1, 1]); loss = sb("loss", [1, 1])
        cpack = sb("cpk", [128, 132])
        onesbf = sb("obf", [128, 1], BF16)

        # --- PSUM: one 16KB tensor, manually laid out ---
        # D ping: chunks at 512*k (448 wide), k=0..3   [banks 0-3]
        # D pong: 2048 + 512*k                          [banks 4-7]
        # qcol ping: [1984:1998) (bank-3 slack); pong: [4032:4046) (bank-7 slack)
        # FT staging (before phase 0): [0:256)
        # finale loss row: [4064:4066) (bank-7 slack)
        PS = ps("PS", [128, 4096])
        Doff = lambda ph, k: (ph % 2) * 2048 + 512 * k
        QOFF = [1984, 4032]
        FOFF = 4064

        # --- semaphores ---
        s_inF = sem("s_inF"); s_inW = sem("s_inW"); s_cst = sem("s_cst")
        s_wtb = [sem(f"s_wtb{i}") for i in range(3)]
        s_qfw = sem("s_qfw"); s_fn = sem("s_fn"); s_wln = sem("s_wln")
        s_sqA = sem("s_sqA"); s_sq = sem("s_sq")
        s_gpA = sem("s_gpA"); s_gp = sem("s_gp"); s_w2s = sem("s_w2s")
        s_qmm = sem("s_qmm"); s_invc = sem("s_invc"); s_fl = sem("s_fl")
        s_invb = sem("s_invb")
        s_D = sem("s_D"); s_mult = sem("s_mult"); s_exp = sem("s_exp")
        s_ftp = sem("s_ftp"); s_ftc = sem("s_ftc")
        s_cy = sem("s_cy"); s_ddf = sem("s_ddf"); s_db = sem("s_db")
        s_mask = sem("s_mask"); s_sin = sem("s_sin"); s_cosm = sem("s_cosm")
        s_eyem = sem("s_eyem")
        s_sb = sem("s_sb"); s_tt = sem("s_tt"); s_lns = sem("s_lns")
        s_li = sem("s_li"); s_fin = sem("s_fin"); s_loss = sem("s_loss")
        s_prep = sem("s_prep"); s_rls = sem("s_rls"); s_rs = sem("s_rs")
        s_cc = sem("s_cc")
        s_gd = sem("s_gd")
        s_vh = sem("s_vh"); s_ah = sem("s_ah")

        _hs = {"v": 0, "a": 0}

        def vbar(eng, ins):
            key = "v" if eng.engine == mybir.EngineType.DVE else "a"
            s = s_vh if key == "v" else s_ah
            _hs[key] += 1
            ins.then_inc(s, 1)
            eng.wait_ge(s, _hs[key])

        with nc.Block() as block:

            # ---------------- SYNC: input DMAs ----------------
            @block.sync
            def _(sync):
                sync.dma_start(
                    F_[:, :, :],
                    bass.AP(feat_ext, 0, [[E, 128], [128 * E, 2], [1, E]]),
                ).then_inc(s_inF, 16)
                sync.dma_start(
                    WL[:, :, :],
                    bass.AP(wlab_ext, 0, [[E, 128], [128 * E, 2], [1, E]]),
                ).then_inc(s_inW, 16)
                sync.dma_start(cpack[:, :], cpack_ext.ap()).then_inc(s_cst, 16)
                sync.dma_start(onesbf[:, :], cbf_ext.ap()).then_inc(s_cst, 16)
                for t in range(NTILES):
                    if t >= 3:
                        sync.wait_ge(s_sq, t - 2)        # ACT squares of t-3 done
                        sync.wait_ge(s_D, 2 * (t - 2))   # PE phases of t-3 done
                    sync.dma_start(
                        WT[t % 3][:, :, :],
                        bass.AP(wt_ext, t * WT_TILE_ELEMS,
                                [[TILE_C, 128], [128 * TILE_C, ECH], [1, TILE_C]]),
                    ).then_inc(s_wtb[t % 3], 16)

            # ---------------- GPSIMD: bcasts, remote exchange, out ----------
            @block.gpsimd
            def _(g):
                g.load_library(library_config.proxy)
                g.collective_compute(
                    "AllGather", mybir.AluOpType.bypass,
                    replica_groups=[list(range(NCORES))],
                    ins=[cc_in.ap().opt()],
                    outs=[cc_out.ap().opt()],
                ).then_inc(s_cc, 1)
                # difficulty row broadcast for the rank/quantile compares
                g.wait_ge(s_cy, 1)
                g.dma_start(
                    bass.AP(DDR, 0, [[B, 1], [2, 128], [1, 2]]),
                    bass.AP(dd, 0, [[2, 128], [1, 2]]),
                ).then_inc(s_ddf, 16)
                g.wait_ge(s_ddf, 16)
                g.partition_broadcast(DB[:, :], DDR[0:1, :]).then_inc(s_db, 1)
                # remote S-exchange descriptors (addresses only; fired at end).
                # Round j sends Sb to peer my_tpb^j into AG slot j; receiver r's
                # slot j ends up with peer r^j's shard sum. Slots are summed, so
                # the sender permutation is irrelevant.
                for j in range(1, NCORES):
                    g.remote_dma_broadcast(
                        AG[:, 2 * j:2 * j + 2], Sb[:, :],
                        remote_sem=s_rs, local_sem=s_rls,
                        rdests=[(0, j) if k == j else None for k in range(NCORES)],
                        queue_num=1,
                    ).then_inc(s_prep, 1)

                # per-tile inv-norm flatten + broadcast
                for t in range(NTILES):
                    g.wait_ge(s_invc, t + 1)
                    g.dma_start(
                        bass.AP(INVR[t % 2], 0, [[TILE_C, 1], [GPT, 128], [1, GPT]]),
                        bass.AP(INVC[t % 2], 0, [[GPT, 128], [GPT, 1], [1, GPT]]),
                    ).then_inc(s_fl, 16)
                    g.wait_ge(s_fl, 16 * (t + 1))
                    if t >= 2:
                        g.wait_ge(s_mult, 2 * (t - 1))  # INVB buffer free
                    g.partition_broadcast(
                        INVB[t % 2][:, :], INVR[t % 2][0:1, :]).then_inc(s_invb, 1)

                # fire the S exchange once local shard sums are ready
                g.wait_ge(s_cc, 1)
                g.wait_ge(s_prep, NCORES - 1)
                g.wait_ge(s_sb, 1)
                g.trigger_dma(count=NCORES - 1, queue_num=1)
                g.wait_ge(s_loss, 1)
                g.dma_start(out_ext[:, :], loss[:, :]).then_inc(s_gd, 16)
                g.wait_ge(s_gd, 16)

            # ---------------- ACT (scalar) ----------------
            @block.scalar
            def _(a):
                Act = mybir.ActivationFunctionType
                halfpi = cpack[:, 129:130]
                neg32 = cpack[:, 130:131]
                ln32c = cpack[:, 131:132]
                # norms of f and wlab (squares -> ln/exp rsqrt, one act table)
                a.wait_ge(s_inF, 16)
                a.activation(CN[0][:, 0:E], F_[:, 0, :], Act.Square,
                             accum_out=qfw[:, 0:1])
                a.activation(CN[0][:, E:2 * E], F_[:, 1, :], Act.Square,
                             accum_out=qfw[:, 1:2])
                a.wait_ge(s_inW, 16)
                a.activation(CN[1][:, 0:E], WL[:, 0, :], Act.Square,
                             accum_out=qfw[:, 2:3])
                ins = a.activation(CN[1][:, E:2 * E], WL[:, 1, :], Act.Square,
                                   accum_out=qfw[:, 3:4])
                vbar(a, ins)
                ins = a.activation(lnq4[:, :], qfw[:, :], Act.Ln)
                vbar(a, ins)
                ins = a.activation(invfw[:, :], lnq4[:, :], Act.Exp, scale=-0.5)
                vbar(a, ins)
                a.activation(FN[:, 0, :], F_[:, 0, :], Act.Copy,
                             scale=invfw[:, 0:1])
                a.activation(FN[:, 1, :], F_[:, 1, :], Act.Copy,
                             scale=invfw[:, 1:2]).then_inc(s_fn, 1)
                a.activation(WLN[:, 0, :], WL[:, 0, :], Act.Copy,
                             scale=invfw[:, 2:3])
                a.activation(WLN[:, 1, :], WL[:, 1, :], Act.Copy,
                             scale=invfw[:, 3:4]).then_inc(s_wln, 1)
                # margin path: cos(m*pi) = -sin(m*pi - pi/2), arg in [0, 1.1]
                a.wait_ge(s_cst, 32)
                a.wait_ge(s_mask, 1)
                a.activation(cmpv[:, :], mm_[:, :], Act.Sin,
                             bias=halfpi, scale=math.pi).then_inc(s_sin, 1)
                a.wait_ge(s_cosm, 1)
                a.activation(ey[:, :], cosy[:, :], Act.Exp,
                             bias=neg32, scale=SCALE)
                a.activation(em[:, :], cosm[:, :], Act.Exp,
                             bias=neg32, scale=SCALE).then_inc(s_eyem, 1)

                def a_square(t):
                    a.wait_ge(s_wtb[t % 3], 16 * (t // 3 + 1))
                    if t >= 1:
                        a.wait_ge(s_gpA, t)       # presum1(t-1) done with W2[0:2]
                    a.activation(W2[:, 0:2, :],
                                 WT[t % 3][:, 0:2, :].bitcast(F32),
                                 Act.Square).then_inc(s_sqA, 1)
                    if t >= 1:
                        a.wait_ge(s_gp, t)        # presum2(t-1) done with W2[2:4]
                    a.activation(W2[:, 2:4, :],
                                 WT[t % 3][:, 2:4, :].bitcast(F32),
                                 Act.Square).then_inc(s_sq, 1)

                def a_inv(t):
                    a.wait_ge(s_qmm, t + 1)
                    if t >= 2:
                        a.wait_ge(s_fl, 16 * (t - 1))  # INVC buffer free
                    ins = a.activation(LNQ[:, :],
                                       PS[:, QOFF[t % 2]:QOFF[t % 2] + GPT],
                                       Act.Ln)
                    vbar(a, ins)
                    a.activation(INVC[t % 2][:, :], LNQ[:, :], Act.Exp,
                                 bias=ln32c, scale=-0.5).then_inc(s_invc, 1)

                def a_exp(ph):
                    t, half = ph // 2, ph % 2
                    a.wait_ge(s_mult, ph + 1)
                    if ph >= 2:
                        a.wait_ge(s_exp, ph - 1)  # EJ buffer visible-order
                    a.activation(
                        EJ[ph % 2][:, :], CN[ph % 2][:, :], Act.Exp,
                        bias=neg32,
                        accum_out=bass.AP(
                            SACC, half * NTILES + t,
                            [[2 * NTILES, 128], [1, 1]])).then_inc(s_exp, 1)

                for t in range(NTILES):
                    a_square(t)
                    if t >= 1:
                        a_exp(2 * (t - 1))
                        a_exp(2 * (t - 1) + 1)
                    a_inv(t)
                a_exp(2 * (NTILES - 1))
                a_exp(2 * (NTILES - 1) + 1)

                # finale
                a.wait_ge(s_tt, 1)
                a.activation(lS[:, :], TT[:, :], Act.Ln).then_inc(s_lns, 1)

            # ---------------- DVE (vector) ----------------
            @block.vector
            def _(v):
                Alu = mybir.AluOpType
                # fT chunk copies (ping-pong with PE transposes through PS[0:256))
                for ec in range(ECH):
                    v.wait_ge(s_ftp, ec + 1)
                    v.tensor_copy(
                        bass.AP(FT, ec * 256, [[ECH * 256, 128], [1, 256]]),
                        PS[:, 0:256]).then_inc(s_ftc, 1)
                # cos_y (exact fp32) and difficulty
                v.wait_ge(s_wln, 1)
                for b in range(2):
                    scrd = CN[b][:, 2 * E:3 * E]  # (128, 512) scratch
                    ins = v.tensor_mul(scrd, FN[:, b, :], WLN[:, b, :])
                    vbar(v, ins)
                    ins = v.tensor_reduce(cosy[:, b:b + 1], scrd,
                                          axis=mybir.AxisListType.X, op=Alu.add)
                    vbar(v, ins)
                v.tensor_scalar(dd[:, :], cosy[:, :], -1.0, 1.0,
                                Alu.mult, Alu.add).then_inc(s_cy, 1)
                # rank/quantile: cnt_i = #{j: d_j <= d_i}; mask = cnt >= 52
                v.wait_ge(s_db, 1)
                for b in range(2):
                    ins = v.tensor_scalar(
                        CMP2[:, :], DB[:, :], dd[:, b:b + 1], 0.0,
                        Alu.is_le, Alu.add, accum_out=cnt[:, b:b + 1])
                    vbar(v, ins)
                v.tensor_scalar(mask[:, :], cnt[:, :], 51.5, None, Alu.is_ge)
                ins = v.tensor_scalar(t1[:, :], dd[:, :], ALPHA, M_BASE,
                                      Alu.mult, Alu.add)
                vbar(v, ins)
                ins = v.tensor_scalar(m015[:, :], mask[:, :], BETA, None, Alu.mult)
                vbar(v, ins)
                v.tensor_add(mm_[:, :], t1[:, :], m015[:, :]).then_inc(s_mask, 1)
                v.wait_ge(s_sin, 1)
                v.tensor_mul(t2[:, :], cosy[:, :], cmpv[:, :])
                ins = v.tensor_scalar(t3[:, :], mm_[:, :], -SIN_M, None, Alu.mult)
                vbar(v, ins)
                v.tensor_sub(cosm[:, :], t3[:, :], t2[:, :]).then_inc(s_cosm, 1)
                v.wait_ge(s_cosm, 1)  # self-wait doubles as visibility barrier
                v.tensor_scalar(li0c[:, :], cosm[:, :], -SCALE, SCALE,
                                Alu.mult, Alu.add)
                v.wait_ge(s_eyem, 1)
                v.tensor_sub(adj[:, :], em[:, :], ey[:, :])

                # big loop: presums interleaved with phase multiplies
                def v_mult(ph):
                    t = ph // 2
                    v.wait_ge(s_D, ph + 1)
                    v.wait_ge(s_invb, t + 1)
                    if ph >= 2:
                        v.wait_ge(s_exp, ph - 1)   # CN buffer free
                    ins = None
                    for k in range(NCH):
                        ins = v.tensor_mul(
                            CN[ph % 2][:, k * CHW:(k + 1) * CHW],
                            PS[:, Doff(ph, k):Doff(ph, k) + CHW],
                            INVB[t % 2][:, k * CHW:(k + 1) * CHW])
                    ins.then_inc(s_mult, 1)

                for t in range(NTILES):
                    v.wait_ge(s_sqA, t + 1)
                    if t >= 1:
                        v.wait_ge(s_w2s, t)       # presum3(t-1) done with W2P
                    v.tensor_add(W2P[:, 0, :], W2[:, 0, :],
                                 W2[:, 1, :]).then_inc(s_gpA, 1)
                    v.wait_ge(s_sq, t + 1)
                    v.tensor_add(W2P[:, 1, :], W2[:, 2, :],
                                 W2[:, 3, :]).then_inc(s_gp, 1)
                    v.wait_ge(s_gp, t + 1)        # barrier: W2P halves visible
                    if t >= 2:
                        v.wait_ge(s_qmm, t - 1)   # W2S buffer free
                    v.tensor_add(W2S[t % 2][:, :], W2P[:, 0, :],
                                 W2P[:, 1, :]).then_inc(s_w2s, 1)
                    v_mult(2 * t)
                    v_mult(2 * t + 1)

                # finale: local shard sum, remote exchange, loss
                v.wait_ge(s_exp, NPH)
                ins = v.tensor_reduce(
                    Sb[:, :],
                    bass.AP(SACC, 0, [[2 * NTILES, 128], [NTILES, 2], [1, NTILES]]),
                    axis=mybir.AxisListType.X, op=Alu.add)
                ins.then_inc(s_sb, 1)
                v.wait_ge(s_rs, 2 * (NCORES - 1))   # all 7 peers' slots landed
                ins = v.tensor_reduce(
                    Sf1[:, :],
                    bass.AP(AG, 2, [[2 * NCORES, 128], [1, 2], [2, NCORES - 1]]),
                    axis=mybir.AxisListType.X, op=Alu.add)
                vbar(v, ins)
                ins = v.tensor_add(Sfull[:, :], Sf1[:, :], Sb[:, :])
                vbar(v, ins)
                v.tensor_add(TT[:, :], Sfull[:, :], adj[:, :]).then_inc(s_tt, 1)
                v.wait_ge(s_lns, 1)
                v.tensor_add(li[:, :], lS[:, :], li0c[:, :]).then_inc(s_li, 1)
                v.wait_ge(s_fin, 1)
                ins = v.tensor_reduce(lsum[:, :], PS[0:1, FOFF:FOFF + 2],
                                      axis=mybir.AxisListType.X, op=Alu.add)
                vbar(v, ins)
                v.tensor_scalar(loss[:, :], lsum[:, :], 1.0 / B, None,
                                Alu.mult).then_inc(s_loss, 1)

            # ---------------- PE (tensor) ----------------
            @block.tensor
            def _(te):
                te.wait_ge(s_cst, 32)
                te.wait_ge(s_fn, 1)
                ident = cpack[:, 0:128]
                onesf = cpack[:, 128:129]
                # fT = transpose(f_norm): [e_p, ec, bc, b] via PS[0:256)
                for ec in range(ECH):
                    if ec >= 1:
                        te.wait_ge(s_ftc, ec)
                    te.transpose(PS[:, 0:128],
                                 FN[:, 0, ec * 128:(ec + 1) * 128], ident)
                    te.transpose(PS[:, 128:256],
                                 FN[:, 1, ec * 128:(ec + 1) * 128],
                                 ident).then_inc(s_ftp, 1)

                def t_qmm(t):
                    # norm matmuls: q_g = ones^T @ W2S strided slice (bf16 FWL)
                    te.wait_ge(s_w2s, t + 1)
                    if t >= 2:
                        te.wait_ge(s_invc, t - 1)   # qcol slack reuse
                    ins = None
                    for gi in range(GPT):
                        # strided class slice {GPT*p + gi}: column p of the
                        # norm output is class GPT*p+gi, so the (p,g) flatten
                        # lands in natural class order
                        ins = te.matmul(
                            PS[:, QOFF[t % 2] + gi:QOFF[t % 2] + gi + 1],
                            bass.AP(W2S[t % 2], gi, [[TILE_C, 128], [GPT, 128]]),
                            onesbf[:, :])
                    ins.then_inc(s_qmm, 1)

                def t_phase(ph):
                    t, half = ph // 2, ph % 2
                    if half == 0:
                        te.wait_ge(s_wtb[t % 3], 16 * (t // 3 + 1))
                    if ph >= 2:
                        te.wait_ge(s_mult, ph - 1)  # D bank-set free
                    ins = None
                    for ec in range(ECH):
                        for k in range(NCH):
                            ins = te.matmul(
                                PS[:, Doff(ph, k):Doff(ph, k) + CHW],
                                FT[:, ec, half, :],
                                WT[t % 3][:, ec, k * CHW:(k + 1) * CHW],
                                start=(ec == 0), stop=(ec == ECH - 1),
                                skip_group_check=True)
                    ins.then_inc(s_D, 1)

                for t in range(NTILES):
                    if t == 0:
                        te.wait_ge(s_ftc, ECH)
                        t_phase(0)
                        t_phase(1)
                        t_qmm(0)
                    else:
                        t_phase(2 * t)
                        t_qmm(t)
                        t_phase(2 * t + 1)

                # finale: batch mean via ones-stationary matmul into bank-7 slack
                te.wait_ge(s_li, 1)
                te.matmul(PS[0:1, FOFF:FOFF + 2], onesf, li[:, :]).then_inc(s_fin, 1)

        return nc


def _shard_host(features, labels, kernel_w):
    """Host-side shard + pack (layout only, no arithmetic)."""
    features = np.ascontiguousarray(features, dtype=np.float32)
    kernel_w = np.ascontiguousarray(kernel_w, dtype=np.float32)
    labels = np.asarray(labels).astype(np.int64)
    wlab = np.ascontiguousarray(kernel_w[labels])        # (B, E) gather
    pad_row = -features[0]                               # direction only matters
    cpack = np.zeros((128, 132), dtype=np.float32)
    cpack[:, 0:128] = np.eye(128, dtype=np.float32)
    cpack[:, 128] = 1.0
    cpack[:, 129] = -math.pi / 2.0
    cpack[:, 130] = -SCALE
    cpack[:, 131] = LN32
    cbf = np.ones((128, 1), dtype=np.float32)  # cast to bf16 below
    try:
        import ml_dtypes
        cbf = cbf.astype(ml_dtypes.bfloat16)
    except ImportError:
        cbf = cbf.astype(np.uint16)  # fallback: bf16(1.0) = 0x3F80
        cbf[:] = 0x3F80
    in_maps = []
    cpc_raw = C // NCORES                                # 12500
    for c in range(NCORES):
        shard = kernel_w[c * cpc_raw:(c + 1) * cpc_raw]  # (12500, E)
        pad = np.broadcast_to(pad_row, (CPC - cpc_raw, E))
        shard = np.concatenate([shard, pad], axis=0)     # (12544, E)
        # (CPC, E) -> transpose -> (E, CPC) -> (ECH,128, NTILES,TILE_C)
        wt = shard.T.reshape(ECH, 128, NTILES, TILE_C)
        wt = np.ascontiguousarray(wt.transpose(2, 0, 1, 3))  # (NTILES,ECH,128,TILE_C)
        in_maps.append({"wt": wt, "feat": features, "wlab": wlab,
                        "cpack": cpack, "cbf": cbf})
    return in_maps


def _get_nc():
    if "nc" not in _CACHE:
        nc = build_nc()
        # populate .instr bytes for extended-inst InstISA ops
        # (partition_broadcast / remote_dma_broadcast / trigger_dma);
        # raw Bass doesn't run this pass and walrus errors on empty .instr.
        from concourse.library_overlay import lower_extended_insts
        lower_extended_insts(nc)
        _CACHE["nc"] = nc
    return _CACHE["nc"]


def kernel(features, labels, kernel):
    in_maps = _shard_host(features, labels, kernel)
    nc = _get_nc()
    res = run_bass_kernel_spmd(nc, in_maps, core_ids=list(range(NCORES)))
    out = res.results[0]["out"]
    return np.float32(out.reshape(())[()])
